# revision 2
# baseline (speedup 1.0000x reference)
"""Multi-head attention (B=4, S=2048, E=768, H=12) on 8 trn2 NeuronCores.

Sharding: 2-D (batch x head-half). Core c handles batch c//2, heads
(c%2)*6 .. (c%2)*6+5  (Wq/Wk/Wv column-split, Wo row-split). Each core
returns a partial O^T [768, S]; host sums the two head-halves per batch,
transposes, and adds the effective output bias (bo + bv@Wo — softmax rows
sum to 1, so V's bias contributes a constant row folded on the host).

Device kernel (per core), bf16 matmuls + fp32 PSUM:
  - masked keys are compacted away on host; padded keys get -30000 added
    via the exp's per-partition bias -> exp == 0.
  - scores/ctx computed transposed (S^T tiles [128 k, q]) so P^T feeds the
    context matmul directly; V carries an appended ones column so row 64
    of the context accumulator is the softmax denominator.
  - the scalar (ACT) engine runs ONLY the exps, one [128, 1024] tile per
    (head, key-chunk); every PSUM evacuation / bias add runs on the
    vector engine so ACT never stalls the softmax.
  - normalization: denominator rows collected (DMA) into 32-aligned rows
    of collector tiles, reciprocal_approx_fast (batched), hi/lo bf16
    split, ones-outer-product broadcast matmul (K=2, exact), multiply.
  - O-projection is interleaved into the attention stream (deferred
    queue) per query block, sharing a 1-bank PSUM tag with the
    normalization broadcasts; consecutive matmuls alternate PSUM banks.
"""

import os
import numpy as np
import ml_dtypes

E = 768
H = 12
D = 64
HALF = 384  # E // 2 output cols per head-half
N_CORES = 8
QB = 1024   # query block (exp tile free dim)

_CACHE = {}
_LAST = None  # last BassKernelResults (for test harness introspection)

bf16_np = ml_dtypes.bfloat16


def _build(S_q, S_pad):
    from contextlib import ExitStack
    import concourse.bass as bass
    import concourse.tile as tile
    from concourse import bacc, mybir

    bf16 = mybir.dt.bfloat16
    f32 = mybir.dt.float32
    FT = mybir.ActivationFunctionType

    NKC = S_pad // 128
    NMC = HALF // 128        # 3 proj-dim chunks (head pairs)
    NEC = E // 128           # 6 embed chunks
    NQB = S_q // QB          # query blocks
    NU = 6 * NQB             # normalization units (head x qblock)
    GS = 3                   # units per collector tile (rows 0/32/64)
    NG = NU // GS

    def ntiles(total, step=512):
        return [(s, min(step, total - s)) for s in range(0, total, step)]

    nc = bacc.Bacc("TRN2", target_bir_lowering=False, debug=False,
                   num_devices=N_CORES)

    qT = nc.dram_tensor("qT", [E, S_q], bf16, kind="ExternalInput").ap()
    kT = nc.dram_tensor("kT", [E, S_pad], bf16, kind="ExternalInput").ap()
    vT = nc.dram_tensor("vT", [E, S_pad], bf16, kind="ExternalInput").ap()
    wq = nc.dram_tensor("wq", [E, HALF], bf16, kind="ExternalInput").ap()
    wk = nc.dram_tensor("wk", [E, HALF], bf16, kind="ExternalInput").ap()
    wv = nc.dram_tensor("wv", [E, HALF], bf16, kind="ExternalInput").ap()
    wo = nc.dram_tensor("wo", [HALF, E], bf16, kind="ExternalInput").ap()
    bq2 = nc.dram_tensor("bq2", [128, NMC], f32, kind="ExternalInput").ap()
    bk2 = nc.dram_tensor("bk2", [128, NMC], f32, kind="ExternalInput").ap()
    kbias = nc.dram_tensor("kbias", [128, NKC], f32, kind="ExternalInput").ap()
    oT = nc.dram_tensor("oT", [E, S_q], f32, kind="ExternalOutput").ap()

    with tile.TileContext(nc) as tc, ExitStack() as ctx:
        cons = ctx.enter_context(tc.tile_pool(name="cons", bufs=1))
        wp = ctx.enter_context(tc.tile_pool(name="wp", bufs=1))
        acts = ctx.enter_context(tc.tile_pool(name="acts", bufs=1))
        pp = ctx.enter_context(tc.tile_pool(name="pp", bufs=3))
        ost = ctx.enter_context(tc.tile_pool(name="ost", bufs=4))
        nrm = ctx.enter_context(tc.tile_pool(name="nrm", bufs=1))

        # ---- constant/small loads ----
        bq2_t = cons.tile([128, NMC], f32, tag="bq2")
        bk2_t = cons.tile([128, NMC], f32, tag="bk2")
        kb_t = cons.tile([128, NKC], f32, tag="kb")
        ones2 = cons.tile([2, 64], bf16, tag="ones2")
        nc.sync.dma_start(bq2_t[:], bq2[:])
        nc.sync.dma_start(bk2_t[:], bk2[:])
        nc.sync.dma_start(kb_t[:], kbias[:])
        nc.vector.memset(ones2[:], 1.0)

        # ---- weight + input loads (inputs in a scoped pool, freed after proj)
        qkv = tc.tile_pool(name="qkv", bufs=1)
        inp = qkv.__enter__()
        wq_t = [wp.tile([128, HALF], bf16, tag=f"wq{e}", name=f"wq{e}") for e in range(NEC)]
        wk_t = [wp.tile([128, HALF], bf16, tag=f"wk{e}", name=f"wk{e}") for e in range(NEC)]
        wv_t = [wp.tile([128, HALF], bf16, tag=f"wv{e}", name=f"wv{e}") for e in range(NEC)]
        wo_t = [wp.tile([128, E], bf16, tag=f"wo{m}", name=f"wo{m}") for m in range(NMC)]
        kT_t = [inp.tile([128, S_pad], bf16, tag=f"kT{e}", name=f"kTt{e}") for e in range(NEC)]
        vT_t = [inp.tile([128, S_pad], bf16, tag=f"vT{e}", name=f"vTt{e}") for e in range(NEC)]
        qT_t = [inp.tile([128, S_q], bf16, tag=f"qT{e}", name=f"qTt{e}") for e in range(NEC)]
        for e in range(NEC):
            nc.sync.dma_start(wk_t[e][:], wk[128 * e:128 * (e + 1), :])
            nc.sync.dma_start(kT_t[e][:], kT[128 * e:128 * (e + 1), :])
        for e in range(NEC):
            nc.sync.dma_start(wv_t[e][:], wv[128 * e:128 * (e + 1), :])
            nc.sync.dma_start(vT_t[e][:], vT[128 * e:128 * (e + 1), :])
        for e in range(NEC):
            nc.sync.dma_start(wq_t[e][:], wq[128 * e:128 * (e + 1), :])
            nc.sync.dma_start(qT_t[e][:], qT[128 * e:128 * (e + 1), :])
        for m in range(NMC):
            nc.sync.dma_start(wo_t[m][:], wo[128 * m:128 * (m + 1), :])

        # ---- projections ----
        kts = [acts.tile([128, S_pad], bf16, tag=f"kts{m}", name=f"kts{m}") for m in range(NMC)]
        qts = [acts.tile([128, S_q], bf16, tag=f"qts{m}", name=f"qts{m}") for m in range(NMC)]
        vhx = [acts.tile([128, 6, 65], bf16, tag=f"vhx{j}", name=f"vhx{j}") for j in range(NKC)]

        psp = tc.tile_pool(name="psp", bufs=1, space="PSUM")
        ps = psp.__enter__()

        def proj_kq(wt, xt, out, bias_t, total):
            # out^T[m-chunk, n]; all n-tiles of one m-chunk accumulate
            # simultaneously so the stationary weight chunk is reused
            # across them (one LDWEIGHTS per (m, e)); consecutive matmuls
            # hit different PSUM banks.
            for m in range(NMC):
                tiles = ntiles(total)
                pjs = [ps.tile([128, 512], f32, tag=f"pj{i}", bufs=2,
                               name=f"pj_{m}_{i}")
                       for i in range(len(tiles))]
                for e in range(NEC):
                    for i, (n0, nw) in enumerate(tiles):
                        nc.tensor.matmul(
                            pjs[i][:, :nw],
                            wt[e][:, 128 * m:128 * (m + 1)],
                            xt[e][:, n0:n0 + nw],
                            start=(e == 0), stop=(e == NEC - 1))
                for i, (n0, nw) in enumerate(tiles):
                    nc.vector.tensor_scalar_add(out[m][:, n0:n0 + nw],
                                                pjs[i][:, :nw],
                                                bias_t[:, m:m + 1])

        proj_kq(wk_t, kT_t, kts, bk2_t, S_pad)

        # V projection: natural layout, alternate PSUM banks via tags
        for j in range(NKC):
            pv = ps.tile([128, 512], f32, tag=f"pj{j % 2}", bufs=2,
                         name=f"pv{j}")
            for e in range(NEC):
                nc.tensor.matmul(pv[:, 0:HALF],
                                 vT_t[e][:, 128 * j:128 * (j + 1)],
                                 wv_t[e][:],
                                 start=(e == 0), stop=(e == NEC - 1))
            nc.vector.tensor_copy(vhx[j][:, :, 0:64],
                                  pv[:, 0:HALF].rearrange("p (h d) -> p h d", h=6))
            nc.vector.memset(vhx[j][:, :, 64:65], 1.0)

        proj_kq(wq_t, qT_t, qts, bq2_t, S_q)
        psp.__exit__(None, None, None)
        qkv.__exit__(None, None, None)

        # ---- attention ----
        czT = [acts.tile([128, S_q], bf16, tag=f"czT{m}", name=f"czT{m}") for m in range(NMC)]
        den_t = [nrm.tile([65, QB], f32, tag=f"den{t}", name=f"den{t}")
                 for t in range(NG)]
        for t in range(NG):
            nc.vector.memset(den_t[t][:], 1.0)

        psa = tc.tile_pool(name="psa", bufs=1, space="PSUM")
        ps = psa.__enter__()

        deferred = []
        hilo = {}
        group_units = {}

        def make_group(t):
            def group():
                recq = nrm.tile([65, QB], f32, tag="recq", bufs=2,
                                name=f"recq{t}")
                nc.vector.reciprocal_approx_fast(recq[:], den_t[t][:])
                hi_t = nrm.tile([65, QB], bf16, tag="hi", bufs=2,
                                name=f"hi{t}")
                lo_t = nrm.tile([65, QB], bf16, tag="lo", bufs=2,
                                name=f"lo{t}")
                nc.vector.tensor_copy(hi_t[:], recq[:])
                nc.vector.tensor_sub(lo_t[:], recq[:], hi_t[:])
                hilo[t] = (hi_t, lo_t)
            return group

        def make_unit(u, cs):
            qb, h = divmod(u, 6)
            m, half = divmod(h, 2)
            t, r = divmod(u, GS)

            def unit():
                hi_t, lo_t = hilo[t]
                hl = nrm.tile([2, QB], bf16, tag="hl", bufs=4, name=f"hl{u}")
                nc.sync.dma_start(hl[0:1, :], hi_t[32 * r:32 * r + 1, :])
                nc.sync.dma_start(hl[1:2, :], lo_t[32 * r:32 * r + 1, :])
                for (t0, tw) in ntiles(QB):
                    bcp = ps.tile([128, 512], f32, tag="aux", bufs=2,
                                  name=f"bcp{u}_{t0}")
                    nc.tensor.matmul(bcp[0:64, :tw], ones2[:],
                                     hl[:, t0:t0 + tw], start=True, stop=True)
                    nc.vector.tensor_mul(
                        czT[m][64 * half:64 * (half + 1),
                               qb * QB + t0:qb * QB + t0 + tw],
                        cs[0:64, t0:t0 + tw], bcp[0:64, :tw])
            return unit

        def make_oproj(qb):
            # one fn per e-chunk: a pair of 512-col PSUM tiles, matmuls
            # interleaved so consecutive matmuls alternate banks.
            fns = []
            for ec in range(NEC):
                def fn(ec=ec, qb=qb):
                    pair = ntiles(QB)
                    pos = [ps.tile([128, 512], f32, tag="aux", bufs=2,
                                   name=f"po{qb}_{ec}_{j}")
                           for j in range(len(pair))]
                    for mm in range(NMC):
                        for j, (t0, tw) in enumerate(pair):
                            nc.tensor.matmul(
                                pos[j][:, :tw],
                                wo_t[mm][:, 128 * ec:128 * (ec + 1)],
                                czT[mm][:, qb * QB + t0:qb * QB + t0 + tw],
                                start=(mm == 0), stop=(mm == NMC - 1))
                    for j, (t0, tw) in enumerate(pair):
                        ot = ost.tile([128, 512], f32, tag="ot",
                                      name=f"ot{qb}_{ec}_{j}")
                        nc.vector.tensor_copy(ot[:, :tw], pos[j][:, :tw])
                        nc.sync.dma_start(
                            oT[128 * ec:128 * (ec + 1),
                               qb * QB + t0:qb * QB + t0 + tw],
                            ot[:, :tw])
                fns.append(fn)
            return fns

        def evac(u, C):
            cs = nrm.tile([65, QB], f32, tag="cs", bufs=6, name=f"cs{u}")
            nc.vector.tensor_copy(cs[:], C[0:65, :])
            t, r = divmod(u, GS)
            nc.sync.dma_start(den_t[t][32 * r:32 * r + 1, :], cs[64:65, :])
            group_units.setdefault(t, []).append(make_unit(u, cs))
            if r == GS - 1:
                deferred.append(make_group(t))
                deferred.extend(group_units.pop(t))
                if u % 6 == 5:  # last unit of this query block
                    deferred.extend(make_oproj(u // 6))

        pend = None
        for qb in range(NQB):
            q0 = qb * QB
            for h in range(6):
                p, half = divmod(h, 2)
                u = qb * 6 + h
                C = ps.tile([128, QB], f32, tag="C", bufs=1, name=f"C{u}")
                if pend is not None:
                    evac(*pend)
                    pend = None

                def s_mm(kc, St):
                    for (t0, tw) in ntiles(QB):
                        nc.tensor.matmul(
                            St[:, t0:t0 + tw],
                            kts[p][64 * half:64 * (half + 1),
                                   128 * kc:128 * (kc + 1)],
                            qts[p][64 * half:64 * (half + 1),
                                   q0 + t0:q0 + t0 + tw],
                            start=True, stop=True,
                            tile_position=(64 * half, 0))

                S = ps.tile([128, QB], f32, tag="S", bufs=2, name=f"S{u}_0")
                s_mm(0, S)
                for kc in range(NKC):
                    if kc >= 1 and deferred:
                        deferred.pop(0)()
                    S2 = None
                    if kc + 1 < NKC:
                        S2 = ps.tile([128, QB], f32, tag="S", bufs=2,
                                     name=f"S{u}_{kc + 1}")
                        s_mm(kc + 1, S2)
                    P = pp.tile([128, QB], bf16, tag="P", name=f"P{u}_{kc}")
                    nc.scalar.activation(P[:], S[:], FT.Exp,
                                         bias=kb_t[:, kc:kc + 1], scale=1.0)
                    for (t0, tw) in ntiles(QB):
                        nc.tensor.matmul(C[0:65, t0:t0 + tw],
                                         vhx[kc][:, h, :], P[:, t0:t0 + tw],
                                         start=(kc == 0), stop=(kc == NKC - 1))
                    S = S2
                pend = (u, C)

        evac(*pend)
        while deferred:
            deferred.pop(0)()
        psa.__exit__(None, None, None)

    nc.compile()
    return nc


def _numpy_fallback(q, k, v, mask, Wq, bq, Wk, bk, Wv, bv, Wo, bo):
    B, Sq, _ = q.shape
    qh = (q @ Wq + bq).reshape(B, Sq, H, D).transpose(0, 2, 1, 3)
    kh = (k @ Wk + bk).reshape(B, -1, H, D).transpose(0, 2, 1, 3)
    vh = (v @ Wv + bv).reshape(B, -1, H, D).transpose(0, 2, 1, 3)
    s = np.einsum("bhqd,bhkd->bhqk", qh, kh) / np.sqrt(np.float32(D))
    s = s + np.where(mask == 0, np.float32(-1e9), np.float32(0))[:, None, None, :]
    s = s - s.max(-1, keepdims=True)
    w = np.exp(s)
    w = w / w.sum(-1, keepdims=True)
    ctx = np.einsum("bhqk,bhkd->bqhd", w, vh).reshape(B, Sq, E)
    return (ctx @ Wo + bo).astype(np.float32)


def kernel(q, k, v, mask, Wq, bq, Wk, bk, Wv, bv, Wo, bo):
    global _LAST
    q = np.asarray(q, np.float32)
    k = np.asarray(k, np.float32)
    v = np.asarray(v, np.float32)
    mask = np.asarray(mask)
    Wq = np.asarray(Wq, np.float32)
    bq = np.asarray(bq, np.float32)
    Wk = np.asarray(Wk, np.float32)
    bk = np.asarray(bk, np.float32)
    Wv = np.asarray(Wv, np.float32)
    bv = np.asarray(bv, np.float32)
    Wo = np.asarray(Wo, np.float32)
    bo = np.asarray(bo, np.float32)

    B, S_q, _ = q.shape
    idxs = [np.flatnonzero(mask[b]) for b in range(B)]
    ns = [len(ix) for ix in idxs]
    if min(ns) == 0 or B * 2 != N_CORES or S_q % QB != 0:
        return _numpy_fallback(q, k, v, mask, Wq, bq, Wk, bk, Wv, bv, Wo, bo)

    S_pad = max(128, ((max(ns) + 127) // 128) * 128)
    NKC = S_pad // 128
    NMC = HALF // 128

    key = (S_q, S_pad)
    if key not in _CACHE:
        _CACHE[key] = _build(S_q, S_pad)
    nc = _CACHE[key]

    scale = np.float32(1.0 / np.sqrt(D))
    in_maps = []
    for c in range(N_CORES):
        b, j = divmod(c, 2)
        cols = slice(j * HALF, (j + 1) * HALF)
        kc_ = np.zeros((S_pad, E), np.float32)
        kc_[:ns[b]] = k[b][idxs[b]]
        vc_ = np.zeros((S_pad, E), np.float32)
        vc_[:ns[b]] = v[b][idxs[b]]
        kb_vec = np.zeros(S_pad, np.float32)
        kb_vec[ns[b]:] = -30000.0
        in_maps.append({
            "qT": np.ascontiguousarray(q[b].T).astype(bf16_np),
            "kT": np.ascontiguousarray(kc_.T).astype(bf16_np),
            "vT": np.ascontiguousarray(vc_.T).astype(bf16_np),
            "wq": (Wq[:, cols] * scale).astype(bf16_np),
            "wk": np.ascontiguousarray(Wk[:, cols]).astype(bf16_np),
            "wv": np.ascontiguousarray(Wv[:, cols]).astype(bf16_np),
            "wo": np.ascontiguousarray(Wo[cols, :]).astype(bf16_np),
            "bq2": np.ascontiguousarray((bq[cols] * scale).reshape(NMC, 128).T),
            "bk2": np.ascontiguousarray(bk[cols].reshape(NMC, 128).T),
            "kbias": np.ascontiguousarray(kb_vec.reshape(NKC, 128).T),
        })

    from concourse.bass_utils import run_bass_kernel_spmd
    res = run_bass_kernel_spmd(nc, in_maps, list(range(N_CORES)))
    _LAST = res

    bo_eff = bo + bv @ Wo
    out = np.empty((B, S_q, E), np.float32)
    for b in range(B):
        out[b] = (res.results[2 * b]["oT"] + res.results[2 * b + 1]["oT"]).T
        out[b] += bo_eff
    return out


# revision 3
# speedup vs baseline: 1.2035x; 1.2035x over previous
"""Multi-head attention (B=4, S=2048, E=768, H=12) on 8 trn2 NeuronCores.

Sharding: 2-D (batch x head-half). Core c handles batch c//2, heads
(c%2)*6 .. (c%2)*6+5  (Wq/Wk/Wv column-split, Wo row-split). Each core
returns a partial O^T [768, S]; host sums the two head-halves per batch,
transposes, and adds the effective output bias (bo + bv@Wo — softmax rows
sum to 1, so V's bias contributes a constant row folded on the host).

Device kernel (per core), bf16 matmuls + fp32 PSUM:
  - masked keys are compacted away on host; padded keys get -30000 added
    via the exp's per-partition bias -> exp == 0.
  - scores/ctx computed transposed (S^T tiles [128 k, q]) so P^T feeds the
    context matmul directly; V carries an appended ones column so row 64
    of the context accumulator is the softmax denominator.
  - the scalar (ACT) engine runs ONLY the exps ([128, 512] single-PSUM-bank
    tiles — multi-bank reads run at half rate); every PSUM evacuation /
    bias add runs on the vector engine.
  - normalization: denominator rows collected (DMA) into 32-aligned rows
    of per-query-block collector tiles, reciprocal_approx_fast (batched),
    hi/lo bf16 split, ones-outer-product broadcast matmul (K=2, exact),
    multiply.
  - O-projection is interleaved into the attention stream (deferred
    queue) per query block, sharing a 1-bank PSUM tag with the
    normalization broadcasts; consecutive matmuls alternate PSUM banks.
"""

import os
import numpy as np
import ml_dtypes

E = 768
H = 12
D = 64
HALF = 384  # E // 2 output cols per head-half
N_CORES = 8
QB = 512    # query block

_CACHE = {}
_LAST = None  # last BassKernelResults (for test harness introspection)

bf16_np = ml_dtypes.bfloat16


def _build(S_q, S_pad):
    from contextlib import ExitStack
    import concourse.bass as bass
    import concourse.tile as tile
    from concourse import bacc, mybir

    bf16 = mybir.dt.bfloat16
    f32 = mybir.dt.float32
    FT = mybir.ActivationFunctionType

    NKC = S_pad // 128
    NMC = HALF // 128        # 3 proj-dim chunks (head pairs)
    NEC = E // 128           # 6 embed chunks
    NQB = S_q // QB          # query blocks
    NU = 6 * NQB             # normalization units (head x qblock)
    GS = 3                   # units per collector tile (rows 0/32/64)
    NG = NU // GS

    def ntiles(total, step=512):
        return [(s, min(step, total - s)) for s in range(0, total, step)]

    nc = bacc.Bacc("TRN2", target_bir_lowering=False, debug=False,
                   num_devices=N_CORES)

    qT = nc.dram_tensor("qT", [E, S_q], bf16, kind="ExternalInput").ap()
    kT = nc.dram_tensor("kT", [E, S_pad], bf16, kind="ExternalInput").ap()
    vT = nc.dram_tensor("vT", [E, S_pad], bf16, kind="ExternalInput").ap()
    wq = nc.dram_tensor("wq", [E, HALF], bf16, kind="ExternalInput").ap()
    wk = nc.dram_tensor("wk", [E, HALF], bf16, kind="ExternalInput").ap()
    wv = nc.dram_tensor("wv", [E, HALF], bf16, kind="ExternalInput").ap()
    wo = nc.dram_tensor("wo", [HALF, E], bf16, kind="ExternalInput").ap()
    bq2 = nc.dram_tensor("bq2", [128, NMC], f32, kind="ExternalInput").ap()
    bk2 = nc.dram_tensor("bk2", [128, NMC], f32, kind="ExternalInput").ap()
    kbias = nc.dram_tensor("kbias", [128, NKC], f32, kind="ExternalInput").ap()
    oT = nc.dram_tensor("oT", [E, S_q], f32, kind="ExternalOutput").ap()

    with tile.TileContext(nc) as tc, ExitStack() as ctx:
        cons = ctx.enter_context(tc.tile_pool(name="cons", bufs=1))
        wp = ctx.enter_context(tc.tile_pool(name="wp", bufs=1))
        acts = ctx.enter_context(tc.tile_pool(name="acts", bufs=1))
        pp = ctx.enter_context(tc.tile_pool(name="pp", bufs=6))
        ost = ctx.enter_context(tc.tile_pool(name="ost", bufs=4))
        nrm = ctx.enter_context(tc.tile_pool(name="nrm", bufs=1))

        # ---- constant/small loads ----
        bq2_t = cons.tile([128, NMC], f32, tag="bq2")
        bk2_t = cons.tile([128, NMC], f32, tag="bk2")
        kb_t = cons.tile([128, NKC], f32, tag="kb")
        ones2 = cons.tile([2, 64], bf16, tag="ones2")
        nc.sync.dma_start(bq2_t[:], bq2[:])
        nc.sync.dma_start(bk2_t[:], bk2[:])
        nc.sync.dma_start(kb_t[:], kbias[:])
        nc.vector.memset(ones2[:], 1.0)

        # ---- weight + input loads (inputs in a scoped pool, freed after proj)
        qkv = tc.tile_pool(name="qkv", bufs=1)
        inp = qkv.__enter__()
        wq_t = [wp.tile([128, HALF], bf16, tag=f"wq{e}", name=f"wq{e}") for e in range(NEC)]
        wk_t = [wp.tile([128, HALF], bf16, tag=f"wk{e}", name=f"wk{e}") for e in range(NEC)]
        wv_t = [wp.tile([128, HALF], bf16, tag=f"wv{e}", name=f"wv{e}") for e in range(NEC)]
        wo_t = [wp.tile([128, E], bf16, tag=f"wo{m}", name=f"wo{m}") for m in range(NMC)]
        kT_t = [inp.tile([128, S_pad], bf16, tag=f"kT{e}", name=f"kTt{e}") for e in range(NEC)]
        vT_t = [inp.tile([128, S_pad], bf16, tag=f"vT{e}", name=f"vTt{e}") for e in range(NEC)]
        qT_t = [inp.tile([128, S_q], bf16, tag=f"qT{e}", name=f"qTt{e}") for e in range(NEC)]
        for e in range(NEC):
            nc.sync.dma_start(wk_t[e][:], wk[128 * e:128 * (e + 1), :])
            nc.sync.dma_start(kT_t[e][:], kT[128 * e:128 * (e + 1), :])
        for e in range(NEC):
            nc.sync.dma_start(wv_t[e][:], wv[128 * e:128 * (e + 1), :])
            nc.sync.dma_start(vT_t[e][:], vT[128 * e:128 * (e + 1), :])
        for e in range(NEC):
            nc.sync.dma_start(wq_t[e][:], wq[128 * e:128 * (e + 1), :])
            nc.sync.dma_start(qT_t[e][:], qT[128 * e:128 * (e + 1), :])
        for m in range(NMC):
            nc.sync.dma_start(wo_t[m][:], wo[128 * m:128 * (m + 1), :])

        # ---- projections ----
        kts = [acts.tile([128, S_pad], bf16, tag=f"kts{m}", name=f"kts{m}") for m in range(NMC)]
        qts = [acts.tile([128, S_q], bf16, tag=f"qts{m}", name=f"qts{m}") for m in range(NMC)]
        vhx = [acts.tile([128, 6, 128], bf16, tag=f"vhx{j}", name=f"vhx{j}") for j in range(NKC)]

        psp = tc.tile_pool(name="psp", bufs=1, space="PSUM")
        ps = psp.__enter__()

        def proj_kq(wt, xt, out, bias_t, total):
            # out^T[m-chunk, n]; all n-tiles of one m-chunk accumulate
            # simultaneously so the stationary weight chunk is reused
            # across them; consecutive matmuls hit different PSUM banks.
            for m in range(NMC):
                tiles = ntiles(total)
                pjs = [ps.tile([128, 512], f32, tag=f"pj{i}", bufs=2,
                               name=f"pj_{m}_{i}")
                       for i in range(len(tiles))]
                for e in range(NEC):
                    for i, (n0, nw) in enumerate(tiles):
                        nc.tensor.matmul(
                            pjs[i][:, :nw],
                            wt[e][:, 128 * m:128 * (m + 1)],
                            xt[e][:, n0:n0 + nw],
                            start=(e == 0), stop=(e == NEC - 1))
                for i, (n0, nw) in enumerate(tiles):
                    nc.vector.tensor_scalar_add(out[m][:, n0:n0 + nw],
                                                pjs[i][:, :nw],
                                                bias_t[:, m:m + 1])

        proj_kq(wk_t, kT_t, kts, bk2_t, S_pad)

        # V projection: natural layout, alternate PSUM banks via tags
        for j in range(NKC):
            pv = ps.tile([128, 512], f32, tag=f"pj{j % 2}", bufs=2,
                         name=f"pv{j}")
            for e in range(NEC):
                nc.tensor.matmul(pv[:, 0:HALF],
                                 vT_t[e][:, 128 * j:128 * (j + 1)],
                                 wv_t[e][:],
                                 start=(e == 0), stop=(e == NEC - 1))
            nc.vector.memset(vhx[j][:, :, 64:128], 1.0)
            nc.vector.tensor_copy(vhx[j][:, :, 0:64],
                                  pv[:, 0:HALF].rearrange("p (h d) -> p h d", h=6))

        proj_kq(wq_t, qT_t, qts, bq2_t, S_q)
        psp.__exit__(None, None, None)
        qkv.__exit__(None, None, None)

        # ---- attention ----
        czT = [acts.tile([128, S_q], bf16, tag=f"czT{m}", name=f"czT{m}") for m in range(NMC)]
        den_t = [nrm.tile([65, QB], f32, tag=f"den{t}", name=f"den{t}")
                 for t in range(NG)]
        for t in range(NG):
            nc.vector.memset(den_t[t][:], 1.0)

        psa = tc.tile_pool(name="psa", bufs=1, space="PSUM")
        ps = psa.__enter__()

        deferred = []
        hilo = {}
        group_units = {}

        def make_group(t):
            def group():
                recq = nrm.tile([65, QB], f32, tag="recq", bufs=2,
                                name=f"recq{t}")
                nc.vector.reciprocal_approx_fast(recq[:], den_t[t][:])
                hi_t = nrm.tile([65, QB], bf16, tag="hi", bufs=2,
                                name=f"hi{t}")
                lo_t = nrm.tile([65, QB], bf16, tag="lo", bufs=2,
                                name=f"lo{t}")
                nc.vector.tensor_copy(hi_t[:], recq[:])
                nc.vector.tensor_sub(lo_t[:], recq[:], hi_t[:])
                hilo[t] = (hi_t, lo_t)
            return group

        def make_unit(u, cs):
            qb, h = divmod(u, 6)
            m, half = divmod(h, 2)
            t, r = divmod(u, GS)

            def unit():
                hi_t, lo_t = hilo[t]
                hl = nrm.tile([2, QB], bf16, tag="hl", bufs=4, name=f"hl{u}")
                nc.sync.dma_start(hl[0:1, :], hi_t[32 * r:32 * r + 1, :])
                nc.sync.dma_start(hl[1:2, :], lo_t[32 * r:32 * r + 1, :])
                bcp = ps.tile([128, QB], f32, tag="aux", bufs=2,
                              name=f"bcp{u}")
                nc.tensor.matmul(bcp[0:64, :], ones2[:], hl[:],
                                 start=True, stop=True)
                nc.vector.tensor_mul(
                    czT[m][64 * half:64 * (half + 1),
                           qb * QB:(qb + 1) * QB],
                    cs[0:64, :], bcp[0:64, :])
            return unit

        def make_oproj(qb):
            # one fn per e-chunk pair; matmuls interleaved so consecutive
            # matmuls alternate the two aux PSUM banks.
            fns = []
            for e0 in range(0, NEC, 2):
                def fn(e0=e0, qb=qb):
                    ecs = (e0, e0 + 1)
                    pos = [ps.tile([128, QB], f32, tag="aux", bufs=2,
                                   name=f"po{qb}_{ec}") for ec in ecs]
                    for mm in range(NMC):
                        for j, ec in enumerate(ecs):
                            nc.tensor.matmul(
                                pos[j][:],
                                wo_t[mm][:, 128 * ec:128 * (ec + 1)],
                                czT[mm][:, qb * QB:(qb + 1) * QB],
                                start=(mm == 0), stop=(mm == NMC - 1))
                    for j, ec in enumerate(ecs):
                        ot = ost.tile([128, QB], f32, tag="ot",
                                      name=f"ot{qb}_{ec}")
                        nc.vector.tensor_copy(ot[:], pos[j][:])
                        nc.sync.dma_start(
                            oT[128 * ec:128 * (ec + 1),
                               qb * QB:(qb + 1) * QB], ot[:])
                fns.append(fn)
            return fns

        def evac(u, C):
            cs = nrm.tile([65, QB], f32, tag="cs", bufs=6, name=f"cs{u}")
            nc.vector.tensor_copy(cs[:], C[0:65, :])
            t, r = divmod(u, GS)
            nc.sync.dma_start(den_t[t][32 * r:32 * r + 1, :], cs[64:65, :])
            group_units.setdefault(t, []).append(make_unit(u, cs))
            if r == GS - 1:
                deferred.append(make_group(t))
                deferred.extend(group_units.pop(t))
                if u % 6 == 5:  # last unit of this query block
                    deferred.extend(make_oproj(u // 6))

        pend_cs = []  # [(u, C), ...] awaiting evacuation
        for qb in range(NQB):
            q0 = qb * QB
            for p in range(NMC):  # head pair: hA=2p (rows 0-63), hB=2p+1
                hA, hB = 2 * p, 2 * p + 1
                CA = ps.tile([128, QB], f32, tag="CA", name=f"CA{qb}_{p}")
                CB = ps.tile([128, QB], f32, tag="CB", name=f"CB{qb}_{p}")
                for pc in pend_cs:
                    evac(*pc)
                pend_cs = []

                def sc_pair(kc, SA_t, SB_t):
                    nc.tensor.matmul(
                        SA_t[:], kts[p][0:64, 128 * kc:128 * (kc + 1)],
                        qts[p][0:64, q0:q0 + QB],
                        start=True, stop=True, tile_position=(0, 0))
                    nc.tensor.matmul(
                        SB_t[:], kts[p][64:128, 128 * kc:128 * (kc + 1)],
                        qts[p][64:128, q0:q0 + QB],
                        start=True, stop=True, tile_position=(64, 0))

                SA = ps.tile([128, QB], f32, tag="SA", bufs=2,
                             name=f"SA{qb}_{p}_0")
                SB = ps.tile([128, QB], f32, tag="SB", bufs=2,
                             name=f"SB{qb}_{p}_0")
                sc_pair(0, SA, SB)
                for kc in range(NKC):
                    if kc >= 1 and deferred:
                        deferred.pop(0)()
                    SA2 = SB2 = None
                    if kc + 1 < NKC:
                        SA2 = ps.tile([128, QB], f32, tag="SA", bufs=2,
                                      name=f"SA{qb}_{p}_{kc + 1}")
                        SB2 = ps.tile([128, QB], f32, tag="SB", bufs=2,
                                      name=f"SB{qb}_{p}_{kc + 1}")
                        sc_pair(kc + 1, SA2, SB2)
                    PA = pp.tile([128, QB], bf16, tag="P", name=f"PA{qb}_{p}_{kc}")
                    PB = pp.tile([128, QB], bf16, tag="P", name=f"PB{qb}_{p}_{kc}")
                    nc.scalar.activation(PA[:], SA[:], FT.Exp,
                                         bias=kb_t[:, kc:kc + 1], scale=1.0)
                    nc.scalar.activation(PB[:], SB[:], FT.Exp,
                                         bias=kb_t[:, kc:kc + 1], scale=1.0)
                    nc.tensor.matmul(CA[:], vhx[kc][:, hA, :], PA[:],
                                     start=(kc == 0), stop=(kc == NKC - 1))
                    nc.tensor.matmul(CB[:], vhx[kc][:, hB, :], PB[:],
                                     start=(kc == 0), stop=(kc == NKC - 1))
                    SA, SB = SA2, SB2
                pend_cs = [(qb * 6 + hA, CA), (qb * 6 + hB, CB)]

        # flush: evacuate last heads, then drain deferred queue
        for pc in pend_cs:
            evac(*pc)
        for fn in deferred:
            fn()
        psa.__exit__(None, None, None)

    nc.compile()
    return nc


def _numpy_fallback(q, k, v, mask, Wq, bq, Wk, bk, Wv, bv, Wo, bo):
    B, Sq, _ = q.shape
    qh = (q @ Wq + bq).reshape(B, Sq, H, D).transpose(0, 2, 1, 3)
    kh = (k @ Wk + bk).reshape(B, -1, H, D).transpose(0, 2, 1, 3)
    vh = (v @ Wv + bv).reshape(B, -1, H, D).transpose(0, 2, 1, 3)
    s = np.einsum("bhqd,bhkd->bhqk", qh, kh) / np.sqrt(np.float32(D))
    s = s + np.where(mask == 0, np.float32(-1e9), np.float32(0))[:, None, None, :]
    s = s - s.max(-1, keepdims=True)
    w = np.exp(s)
    w = w / w.sum(-1, keepdims=True)
    ctx = np.einsum("bhqk,bhkd->bqhd", w, vh).reshape(B, Sq, E)
    return (ctx @ Wo + bo).astype(np.float32)


def kernel(q, k, v, mask, Wq, bq, Wk, bk, Wv, bv, Wo, bo):
    global _LAST
    q = np.asarray(q, np.float32)
    k = np.asarray(k, np.float32)
    v = np.asarray(v, np.float32)
    mask = np.asarray(mask)
    Wq = np.asarray(Wq, np.float32)
    bq = np.asarray(bq, np.float32)
    Wk = np.asarray(Wk, np.float32)
    bk = np.asarray(bk, np.float32)
    Wv = np.asarray(Wv, np.float32)
    bv = np.asarray(bv, np.float32)
    Wo = np.asarray(Wo, np.float32)
    bo = np.asarray(bo, np.float32)

    B, S_q, _ = q.shape
    idxs = [np.flatnonzero(mask[b]) for b in range(B)]
    ns = [len(ix) for ix in idxs]
    if min(ns) == 0 or B * 2 != N_CORES or S_q % QB != 0:
        return _numpy_fallback(q, k, v, mask, Wq, bq, Wk, bk, Wv, bv, Wo, bo)

    S_pad = max(128, ((max(ns) + 127) // 128) * 128)
    NKC = S_pad // 128
    NMC = HALF // 128

    key = (S_q, S_pad)
    if key not in _CACHE:
        _CACHE[key] = _build(S_q, S_pad)
    nc = _CACHE[key]

    scale = np.float32(1.0 / np.sqrt(D))
    in_maps = []
    for c in range(N_CORES):
        b, j = divmod(c, 2)
        cols = slice(j * HALF, (j + 1) * HALF)
        kc_ = np.zeros((S_pad, E), np.float32)
        kc_[:ns[b]] = k[b][idxs[b]]
        vc_ = np.zeros((S_pad, E), np.float32)
        vc_[:ns[b]] = v[b][idxs[b]]
        kb_vec = np.zeros(S_pad, np.float32)
        kb_vec[ns[b]:] = -30000.0
        in_maps.append({
            "qT": np.ascontiguousarray(q[b].T).astype(bf16_np),
            "kT": np.ascontiguousarray(kc_.T).astype(bf16_np),
            "vT": np.ascontiguousarray(vc_.T).astype(bf16_np),
            "wq": (Wq[:, cols] * scale).astype(bf16_np),
            "wk": np.ascontiguousarray(Wk[:, cols]).astype(bf16_np),
            "wv": np.ascontiguousarray(Wv[:, cols]).astype(bf16_np),
            "wo": np.ascontiguousarray(Wo[cols, :]).astype(bf16_np),
            "bq2": np.ascontiguousarray((bq[cols] * scale).reshape(NMC, 128).T),
            "bk2": np.ascontiguousarray(bk[cols].reshape(NMC, 128).T),
            "kbias": np.ascontiguousarray(kb_vec.reshape(NKC, 128).T),
        })

    from concourse.bass_utils import run_bass_kernel_spmd
    res = run_bass_kernel_spmd(nc, in_maps, list(range(N_CORES)))
    _LAST = res

    bo_eff = bo + bv @ Wo
    out = np.empty((B, S_q, E), np.float32)
    for b in range(B):
        out[b] = (res.results[2 * b]["oT"] + res.results[2 * b + 1]["oT"]).T
        out[b] += bo_eff
    return out


# revision 8
# speedup vs baseline: 1.4743x; 1.2250x over previous
"""Multi-head attention (B=4, S=2048, E=768, H=12) on 8 trn2 NeuronCores.

Sharding: 2-D (batch x head-half). Core c handles batch c//2, heads
(c%2)*6 .. (c%2)*6+5  (Wq/Wk/Wv column-split, Wo row-split). Each core
returns a partial O^T [768, S]; host sums the two head-halves per batch,
transposes, and adds the effective output bias (bo + bv@Wo — softmax rows
sum to 1, so V's bias contributes a constant row folded on the host).

Device kernel (per core), bf16 matmuls + fp32 PSUM:
  - masked keys are compacted away on host; padded keys get -30000 added
    via the exp's per-partition bias -> exp == 0.
  - scores/ctx computed transposed (S^T tiles [128 k, q]) so P^T feeds the
    context matmul directly; V carries an appended ones column so row 64
    of the context accumulator is the softmax denominator.
  - the scalar (ACT) engine runs ONLY the exps; each head-pair's two score
    tiles live in one [128, 2, 512] PSUM tile so a single exp covers both
    (halves ACT's per-instruction semaphore overhead).
  - only the m=0 chunk of the K/Q projections (plus all of V) runs before
    attention; the m=1,2 chunks are deferred into the attention stream so
    the PE stays dense (HAM stays at full clock) and the exps start ~20us
    earlier.
  - normalization: denominator rows collected (DMA) into 32-aligned rows
    of per-query-block collector tiles, reciprocal_approx_fast (batched),
    hi/lo bf16 split, ones-outer-product broadcast matmul (K=2, exact),
    multiply. O-projection is likewise interleaved via the deferred queue.
"""

import os
import numpy as np
import ml_dtypes

E = 768
H = 12
D = 64
HALF = 384  # E // 2 output cols per head-half
N_CORES = 8
QB = 512    # query block

_CACHE = {}
_LAST = None  # last BassKernelResults (for test harness introspection)

bf16_np = ml_dtypes.bfloat16


def _build(S_q, S_pad):
    from contextlib import ExitStack
    import concourse.bass as bass
    import concourse.tile as tile
    from concourse import bacc, mybir

    bf16 = mybir.dt.bfloat16
    f32 = mybir.dt.float32
    FT = mybir.ActivationFunctionType

    NKC = S_pad // 128
    NMC = HALF // 128        # 3 proj-dim chunks (head pairs)
    NEC = E // 128           # 6 embed chunks
    NQB = S_q // QB          # query blocks
    NU = 6 * NQB             # normalization units (head x qblock)
    GS = 2                   # units per collector tile (rows 0/32)
    NG = NU // GS

    def ntiles(total, step=512):
        return [(s, min(step, total - s)) for s in range(0, total, step)]

    nc = bacc.Bacc("TRN2", target_bir_lowering=False, debug=False,
                   num_devices=N_CORES)

    qT = nc.dram_tensor("qT", [E, S_q], bf16, kind="ExternalInput").ap()
    kT = nc.dram_tensor("kT", [E, S_pad], bf16, kind="ExternalInput").ap()
    vT = nc.dram_tensor("vT", [E, S_pad], bf16, kind="ExternalInput").ap()
    wq = nc.dram_tensor("wq", [E, HALF], bf16, kind="ExternalInput").ap()
    wk = nc.dram_tensor("wk", [E, HALF], bf16, kind="ExternalInput").ap()
    wv = nc.dram_tensor("wv", [E, HALF], bf16, kind="ExternalInput").ap()
    wo = nc.dram_tensor("wo", [HALF, E], bf16, kind="ExternalInput").ap()
    bq2 = nc.dram_tensor("bq2", [128, NMC], f32, kind="ExternalInput").ap()
    bk2 = nc.dram_tensor("bk2", [128, NMC], f32, kind="ExternalInput").ap()
    kbias = nc.dram_tensor("kbias", [128, NKC], f32, kind="ExternalInput").ap()
    oT = nc.dram_tensor("oT", [E, S_q], f32, kind="ExternalOutput").ap()

    with tile.TileContext(nc) as tc, ExitStack() as ctx:
        cons = ctx.enter_context(tc.tile_pool(name="cons", bufs=1))
        wp = ctx.enter_context(tc.tile_pool(name="wp", bufs=1))
        acts = ctx.enter_context(tc.tile_pool(name="acts", bufs=1))
        pp = ctx.enter_context(tc.tile_pool(name="pp", bufs=6))
        ost = ctx.enter_context(tc.tile_pool(name="ost", bufs=4))
        nrm = ctx.enter_context(tc.tile_pool(name="nrm", bufs=1))

        # ---- constant/small loads ----
        bq2_t = cons.tile([128, NMC], f32, tag="bq2")
        bk2_t = cons.tile([128, NMC], f32, tag="bk2")
        kb_t = cons.tile([128, NKC], f32, tag="kb")
        ones2 = cons.tile([2, 64], bf16, tag="ones2")
        nc.sync.dma_start(bq2_t[:], bq2[:])
        nc.sync.dma_start(bk2_t[:], bk2[:])
        nc.sync.dma_start(kb_t[:], kbias[:])
        nc.vector.memset(ones2[:], 1.0)

        # ---- weight + input loads (inputs freed after deferred proj) ----
        qkv = tc.tile_pool(name="qkv", bufs=1)
        inp = qkv.__enter__()
        wq_t = [wp.tile([128, HALF], bf16, tag=f"wq{e}", name=f"wq{e}") for e in range(NEC)]
        wk_t = [wp.tile([128, HALF], bf16, tag=f"wk{e}", name=f"wk{e}") for e in range(NEC)]
        wv_t = [wp.tile([128, HALF], bf16, tag=f"wv{e}", name=f"wv{e}") for e in range(NEC)]
        wo_t = [wp.tile([128, E], bf16, tag=f"wo{m}", name=f"wo{m}") for m in range(NMC)]
        kT_t = [inp.tile([128, S_pad], bf16, tag=f"kT{e}", name=f"kTt{e}") for e in range(NEC)]
        vT_t = [inp.tile([128, S_pad], bf16, tag=f"vT{e}", name=f"vTt{e}") for e in range(NEC)]
        qT_t = [inp.tile([128, S_q], bf16, tag=f"qT{e}", name=f"qTt{e}") for e in range(NEC)]
        for e in range(NEC):
            nc.sync.dma_start(wv_t[e][:], wv[128 * e:128 * (e + 1), :])
            nc.sync.dma_start(vT_t[e][:], vT[128 * e:128 * (e + 1), :])
        for e in range(NEC):
            nc.sync.dma_start(wk_t[e][:], wk[128 * e:128 * (e + 1), :])
            nc.sync.dma_start(kT_t[e][:], kT[128 * e:128 * (e + 1), :])
        for e in range(NEC):
            nc.sync.dma_start(wq_t[e][:], wq[128 * e:128 * (e + 1), :])
            nc.sync.dma_start(qT_t[e][:], qT[128 * e:128 * (e + 1), :])
        for m in range(NMC):
            nc.sync.dma_start(wo_t[m][:], wo[128 * m:128 * (m + 1), :])

        # ---- prologue projections: V (all), K m=0, Q m=0 ----
        kts = [acts.tile([128, S_pad], bf16, tag=f"kts{m}", name=f"kts{m}") for m in range(NMC)]
        qts = [acts.tile([128, S_q], bf16, tag=f"qts{m}", name=f"qts{m}") for m in range(NMC)]
        vhx = [acts.tile([128, 6, 128], bf16, tag=f"vhx{j}", name=f"vhx{j}") for j in range(NKC)]

        psp = tc.tile_pool(name="psp", bufs=1, space="PSUM")
        ps = psp.__enter__()

        # V projection: natural layout, alternate PSUM banks via tags
        for j in range(NKC):
            pv = ps.tile([128, 512], f32, tag=f"pj{j % 2}", bufs=2,
                         name=f"pv{j}")
            for e in range(NEC):
                nc.tensor.matmul(pv[:, 0:HALF],
                                 vT_t[e][:, 128 * j:128 * (j + 1)],
                                 wv_t[e][:],
                                 start=(e == 0), stop=(e == NEC - 1))
            nc.vector.memset(vhx[j][:, :, 64:128], 1.0)
            nc.vector.tensor_copy(vhx[j][:, :, 0:64],
                                  pv[:, 0:HALF].rearrange("p (h d) -> p h d", h=6))

        def proj_m(psl, tag, bufs, wt, xt, out, bias_t, m, tiles):
            # all n-tiles of the m-chunk accumulate together: stationary
            # reuse across tiles, consecutive matmuls on different banks.
            pjs = [psl.tile([128, 512], f32, tag=tag(i), bufs=bufs,
                            name=f"pj{m}_{tiles[i][0]}_{id(wt)}")
                   for i in range(len(tiles))]
            for e in range(NEC):
                for i, (n0, nw) in enumerate(tiles):
                    nc.tensor.matmul(
                        pjs[i][:, :nw],
                        wt[e][:, 128 * m:128 * (m + 1)],
                        xt[e][:, n0:n0 + nw],
                        start=(e == 0), stop=(e == NEC - 1))
            for i, (n0, nw) in enumerate(tiles):
                nc.vector.tensor_scalar_add(out[m][:, n0:n0 + nw],
                                            pjs[i][:, :nw],
                                            bias_t[:, m:m + 1])

        proj_m(ps, lambda i: f"pj{i}", 2, wk_t, kT_t, kts, bk2_t, 0,
               ntiles(S_pad))
        proj_m(ps, lambda i: f"pj{i}", 2, wq_t, qT_t, qts, bq2_t, 0,
               ntiles(S_q))
        psp.__exit__(None, None, None)

        # ---- attention ----
        czT = [acts.tile([128, S_q], bf16, tag=f"czT{m}", name=f"czT{m}") for m in range(NMC)]
        den_t = [nrm.tile([33, QB], f32, tag=f"den{t}", name=f"den{t}")
                 for t in range(NG)]
        for t in range(NG):
            nc.vector.memset(den_t[t][:], 1.0)

        psa = tc.tile_pool(name="psa", bufs=1, space="PSUM")
        ps = psa.__enter__()

        deferred = []
        hl_t = {}
        group_units = {}
        pushed_units = [0] * NQB

        # deferred m=1,2 K/Q projection chunks, split into e-halves so each
        # pop costs ~1.3us of PE; halves interleave across fns (A1 B1 A2 B2)
        # so the two open accumulations alternate the aux banks.
        def make_proj_halves(wt, xt, out, bias_t, m, pair):
            pjs = []

            def h1():
                pjs.extend(ps.tile([128, 512], f32, tag="aux", bufs=2,
                                   name=f"dpj{m}_{pair[i][0]}_{id(wt)}")
                           for i in range(len(pair)))
                for e in range(NEC // 2):
                    for i, (n0, nw) in enumerate(pair):
                        nc.tensor.matmul(pjs[i][:, :nw],
                                         wt[e][:, 128 * m:128 * (m + 1)],
                                         xt[e][:, n0:n0 + nw],
                                         start=(e == 0), stop=False)

            def h2():
                for e in range(NEC // 2, NEC):
                    for i, (n0, nw) in enumerate(pair):
                        nc.tensor.matmul(pjs[i][:, :nw],
                                         wt[e][:, 128 * m:128 * (m + 1)],
                                         xt[e][:, n0:n0 + nw],
                                         start=False, stop=(e == NEC - 1))
                for i, (n0, nw) in enumerate(pair):
                    nc.vector.tensor_scalar_add(out[m][:, n0:n0 + nw],
                                                pjs[i][:, :nw],
                                                bias_t[:, m:m + 1])
            return h1, h2

        for m in (1, 2):
            halves = []
            for wt, xt, out, bias_t, total in (
                    (wk_t, kT_t, kts, bk2_t, S_pad),
                    (wq_t, qT_t, qts, bq2_t, S_q)):
                tiles = ntiles(total)
                for i0 in range(0, len(tiles), 2):
                    halves.append(make_proj_halves(wt, xt, out, bias_t, m,
                                                   tiles[i0:i0 + 2]))
            for a in halves:
                deferred.extend(a)

        def make_group(t, members):
            def group():
                recq = nrm.tile([33, QB], f32, tag="recq", bufs=2,
                                name=f"recq{t}")
                nc.vector.reciprocal_approx_fast(recq[:], den_t[t][:])
                hi_t = nrm.tile([33, QB], bf16, tag="hi", bufs=2,
                                name=f"hi{t}")
                lo_t = nrm.tile([33, QB], bf16, tag="lo", bufs=2,
                                name=f"lo{t}")
                nc.vector.tensor_copy(hi_t[:], recq[:])
                nc.vector.tensor_sub(lo_t[:], recq[:], hi_t[:])
                # issue the hi/lo row DMAs now so popped PE work below
                # never waits on them at the head of the PE queue.
                for (u, r, _) in members:
                    hl = nrm.tile([2, QB], bf16, tag="hl", bufs=4,
                                  name=f"hl{u}")
                    nc.sync.dma_start(hl[0:1, :], hi_t[32 * r:32 * r + 1, :])
                    nc.sync.dma_start(hl[1:2, :], lo_t[32 * r:32 * r + 1, :])
                    hl_t[u] = hl
            return group

        def make_unit(u, cs):
            qb, h = divmod(u, 6)
            m, half = divmod(h, 2)

            def unit():
                hl = hl_t.pop(u)
                bcp = ps.tile([128, QB], f32, tag="aux", bufs=2,
                              name=f"bcp{u}")
                nc.tensor.matmul(bcp[0:64, :], ones2[:], hl[:],
                                 start=True, stop=True)
                nc.vector.tensor_mul(
                    czT[m][64 * half:64 * (half + 1),
                           qb * QB:(qb + 1) * QB],
                    cs[0:64, :], bcp[0:64, :])
            return unit

        def make_oproj(qb):
            # one fn per e-chunk pair; matmuls interleaved so consecutive
            # matmuls alternate the two aux PSUM banks.
            fns = []
            for e0 in range(0, NEC, 2):
                def fn(e0=e0, qb=qb):
                    ecs = (e0, e0 + 1)
                    pos = [ps.tile([128, QB], f32, tag="aux", bufs=2,
                                   name=f"po{qb}_{ec}") for ec in ecs]
                    for mm in range(NMC):
                        for j, ec in enumerate(ecs):
                            nc.tensor.matmul(
                                pos[j][:],
                                wo_t[mm][:, 128 * ec:128 * (ec + 1)],
                                czT[mm][:, qb * QB:(qb + 1) * QB],
                                start=(mm == 0), stop=(mm == NMC - 1))
                    for j, ec in enumerate(ecs):
                        ot = ost.tile([128, QB], f32, tag="ot",
                                      name=f"ot{qb}_{ec}")
                        nc.vector.tensor_copy(ot[:], pos[j][:])
                        nc.sync.dma_start(
                            oT[128 * ec:128 * (ec + 1),
                               qb * QB:(qb + 1) * QB], ot[:])
                fns.append(fn)
            return fns

        arrival = [0]

        def evac(u, C):
            cs = nrm.tile([65, QB], f32, tag="cs", bufs=6, name=f"cs{u}")
            nc.vector.tensor_copy(cs[:], C[0:65, :])
            a = arrival[0]
            arrival[0] += 1
            t, r = divmod(a, GS)
            nc.sync.dma_start(den_t[t][32 * r:32 * r + 1, :], cs[64:65, :])
            group_units.setdefault(t, []).append((u, r, make_unit(u, cs)))
            if r == GS - 1:
                members = group_units.pop(t)
                deferred.append(make_group(t, [(u_, r_, None)
                                               for (u_, r_, _) in members]))
                for (u_, _, fn) in members:
                    deferred.append(fn)
                    qb_ = u_ // 6
                    pushed_units[qb_] += 1
                    if pushed_units[qb_] == 6:
                        deferred.extend(make_oproj(qb_))

        pend_cs = []  # [(u, C), ...] awaiting evacuation
        for p in range(NMC):      # head pair: hA=2p (rows 0-63), hB=2p+1
            for qb in range(NQB):
                q0 = qb * QB
                hA, hB = 2 * p, 2 * p + 1
                CA = ps.tile([128, QB], f32, tag="CA", name=f"CA{qb}_{p}")
                CB = ps.tile([128, QB], f32, tag="CB", name=f"CB{qb}_{p}")
                for pc in pend_cs:
                    evac(*pc)
                pend_cs = []

                def sc_pair(kc, St):
                    nc.tensor.matmul(
                        St[:, 0, :], kts[p][0:64, 128 * kc:128 * (kc + 1)],
                        qts[p][0:64, q0:q0 + QB],
                        start=True, stop=True, tile_position=(0, 0))
                    nc.tensor.matmul(
                        St[:, 1, :], kts[p][64:128, 128 * kc:128 * (kc + 1)],
                        qts[p][64:128, q0:q0 + QB],
                        start=True, stop=True, tile_position=(64, 0))

                S = ps.tile([128, 2, QB], f32, tag="S", bufs=2,
                            name=f"S{qb}_{p}_0")
                sc_pair(0, S)
                for kc in range(NKC):
                    if kc >= 1 and deferred:
                        deferred.pop(0)()
                        if len(deferred) > 8 and deferred:
                            deferred.pop(0)()
                    S2 = None
                    if kc + 1 < NKC:
                        S2 = ps.tile([128, 2, QB], f32, tag="S", bufs=2,
                                     name=f"S{qb}_{p}_{kc + 1}")
                        sc_pair(kc + 1, S2)
                    P = pp.tile([128, 2, QB], bf16, tag="P",
                                name=f"P{qb}_{p}_{kc}")
                    nc.scalar.activation(P[:], S[:], FT.Exp,
                                         bias=kb_t[:, kc:kc + 1], scale=1.0)
                    nc.tensor.matmul(CA[:], vhx[kc][:, hA, :], P[:, 0, :],
                                     start=(kc == 0), stop=(kc == NKC - 1))
                    nc.tensor.matmul(CB[:], vhx[kc][:, hB, :], P[:, 1, :],
                                     start=(kc == 0), stop=(kc == NKC - 1))
                    S = S2
                pend_cs = [(qb * 6 + hA, CA), (qb * 6 + hB, CB)]

        # flush: evacuate last heads, then drain deferred queue
        for pc in pend_cs:
            evac(*pc)
        for fn in deferred:
            fn()
        psa.__exit__(None, None, None)
        qkv.__exit__(None, None, None)

    nc.compile()
    return nc


def _numpy_fallback(q, k, v, mask, Wq, bq, Wk, bk, Wv, bv, Wo, bo):
    B, Sq, _ = q.shape
    qh = (q @ Wq + bq).reshape(B, Sq, H, D).transpose(0, 2, 1, 3)
    kh = (k @ Wk + bk).reshape(B, -1, H, D).transpose(0, 2, 1, 3)
    vh = (v @ Wv + bv).reshape(B, -1, H, D).transpose(0, 2, 1, 3)
    s = np.einsum("bhqd,bhkd->bhqk", qh, kh) / np.sqrt(np.float32(D))
    s = s + np.where(mask == 0, np.float32(-1e9), np.float32(0))[:, None, None, :]
    s = s - s.max(-1, keepdims=True)
    w = np.exp(s)
    w = w / w.sum(-1, keepdims=True)
    ctx = np.einsum("bhqk,bhkd->bqhd", w, vh).reshape(B, Sq, E)
    return (ctx @ Wo + bo).astype(np.float32)


def kernel(q, k, v, mask, Wq, bq, Wk, bk, Wv, bv, Wo, bo):
    global _LAST
    q = np.asarray(q, np.float32)
    k = np.asarray(k, np.float32)
    v = np.asarray(v, np.float32)
    mask = np.asarray(mask)
    Wq = np.asarray(Wq, np.float32)
    bq = np.asarray(bq, np.float32)
    Wk = np.asarray(Wk, np.float32)
    bk = np.asarray(bk, np.float32)
    Wv = np.asarray(Wv, np.float32)
    bv = np.asarray(bv, np.float32)
    Wo = np.asarray(Wo, np.float32)
    bo = np.asarray(bo, np.float32)

    B, S_q, _ = q.shape
    idxs = [np.flatnonzero(mask[b]) for b in range(B)]
    ns = [len(ix) for ix in idxs]
    if min(ns) == 0 or B * 2 != N_CORES or S_q % QB != 0:
        return _numpy_fallback(q, k, v, mask, Wq, bq, Wk, bk, Wv, bv, Wo, bo)

    S_pad = max(128, ((max(ns) + 127) // 128) * 128)
    NKC = S_pad // 128
    NMC = HALF // 128

    key = (S_q, S_pad)
    if key not in _CACHE:
        _CACHE[key] = _build(S_q, S_pad)
    nc = _CACHE[key]

    scale = np.float32(1.0 / np.sqrt(D))
    in_maps = []
    for c in range(N_CORES):
        b, j = divmod(c, 2)
        cols = slice(j * HALF, (j + 1) * HALF)
        kc_ = np.zeros((S_pad, E), np.float32)
        kc_[:ns[b]] = k[b][idxs[b]]
        vc_ = np.zeros((S_pad, E), np.float32)
        vc_[:ns[b]] = v[b][idxs[b]]
        kb_vec = np.zeros(S_pad, np.float32)
        kb_vec[ns[b]:] = -30000.0
        in_maps.append({
            "qT": np.ascontiguousarray(q[b].T).astype(bf16_np),
            "kT": np.ascontiguousarray(kc_.T).astype(bf16_np),
            "vT": np.ascontiguousarray(vc_.T).astype(bf16_np),
            "wq": (Wq[:, cols] * scale).astype(bf16_np),
            "wk": np.ascontiguousarray(Wk[:, cols]).astype(bf16_np),
            "wv": np.ascontiguousarray(Wv[:, cols]).astype(bf16_np),
            "wo": np.ascontiguousarray(Wo[cols, :]).astype(bf16_np),
            "bq2": np.ascontiguousarray((bq[cols] * scale).reshape(NMC, 128).T),
            "bk2": np.ascontiguousarray(bk[cols].reshape(NMC, 128).T),
            "kbias": np.ascontiguousarray(kb_vec.reshape(NKC, 128).T),
        })

    from concourse.bass_utils import run_bass_kernel_spmd
    res = run_bass_kernel_spmd(nc, in_maps, list(range(N_CORES)))
    _LAST = res

    bo_eff = bo + bv @ Wo
    out = np.empty((B, S_q, E), np.float32)
    for b in range(B):
        out[b] = (res.results[2 * b]["oT"] + res.results[2 * b + 1]["oT"]).T
        out[b] += bo_eff
    return out


# revision 23
# speedup vs baseline: 1.5645x; 1.0612x over previous
"""Multi-head attention (B=4, S=2048, E=768, H=12) on 8 trn2 NeuronCores.

Sharding: 2-D (batch x head-half). Core c handles batch c//2, heads
(c%2)*6 .. (c%2)*6+5  (Wq/Wk/Wv column-split, Wo row-split). Each core
returns a partial O^T [768, S]; host sums the two head-halves per batch,
transposes, and adds the effective output bias (bo + bv@Wo — softmax rows
sum to 1, so V's bias contributes a constant row folded on the host).

Device kernel (per core), bf16 matmuls + fp32 PSUM:
  - masked keys are compacted away on host; padded keys get -30000 added
    via the exp's per-partition bias -> exp == 0.
  - scores/ctx computed transposed (S^T tiles [128 k, q]) so P^T feeds the
    context matmul directly; V carries an appended ones column so row 64
    of the context accumulator is the softmax denominator.
  - the scalar (ACT) engine runs ONLY the exps; each head-pair's two score
    tiles live in one [128, 2, 512] PSUM tile so a single exp covers both.
  - normalization per (head, qblock): DMA the denominator row to
    partition 0, reciprocal there, exact hi/lo bf16 split, and broadcast
    across the 64 context rows with two accumulating K=1 ones-matmuls.
  - inputs stream in deadline order (K, Q-head, V-head, V-tail, Q-tail);
    only K/Q m=0 head-tiles project before attention — everything else
    (V, remaining projections, O-projection, normalization) is deferred
    into the attention stream in ~1us chunks popped once per key-chunk,
    so exps start ~15us in and the PE stays dense (HAM at full clock).
    Attention blocks run head-pair-outer so deferred m=1,2 projections
    have 4 blocks of slack before their consumers.
"""

import os
import numpy as np
import ml_dtypes

E = 768
H = 12
D = 64
HALF = 384  # E // 2 output cols per head-half
N_CORES = 8
QB = 512    # query block

_CACHE = {}
_LAST = None  # last BassKernelResults (for test harness introspection)

bf16_np = ml_dtypes.bfloat16


def _build(S_q, S_pad):
    from contextlib import ExitStack
    import concourse.bass as bass
    import concourse.tile as tile
    from concourse import bacc, mybir

    bf16 = mybir.dt.bfloat16
    f32 = mybir.dt.float32
    FT = mybir.ActivationFunctionType

    NKC = S_pad // 128
    NMC = HALF // 128        # 3 proj-dim chunks (head pairs)
    NEC = E // 128           # 6 embed chunks
    NQB = S_q // QB          # query blocks
    QA = 1024                # qT head split (first two n-tiles)
    VA = 512                 # vT head split (first four key chunks)

    def ntiles(total, step=512):
        return [(s, min(step, total - s)) for s in range(0, total, step)]

    nc = bacc.Bacc("TRN2", target_bir_lowering=False, debug=False,
                   num_devices=N_CORES)

    qTa = nc.dram_tensor("qTa", [E, QA], bf16, kind="ExternalInput").ap()
    qTb = nc.dram_tensor("qTb", [E, S_q - QA], bf16, kind="ExternalInput").ap()
    kT = nc.dram_tensor("kT", [E, S_pad], bf16, kind="ExternalInput").ap()
    vTa = nc.dram_tensor("vTa", [E, VA], bf16, kind="ExternalInput").ap()
    vTb = nc.dram_tensor("vTb", [E, S_pad - VA], bf16, kind="ExternalInput").ap()
    wq = nc.dram_tensor("wq", [E, HALF], bf16, kind="ExternalInput").ap()
    wk = nc.dram_tensor("wk", [E, HALF], bf16, kind="ExternalInput").ap()
    wv = nc.dram_tensor("wv", [E, HALF], bf16, kind="ExternalInput").ap()
    wo = nc.dram_tensor("wo", [HALF, E], bf16, kind="ExternalInput").ap()
    bq2 = nc.dram_tensor("bq2", [128, NMC], f32, kind="ExternalInput").ap()
    bk2 = nc.dram_tensor("bk2", [128, NMC], f32, kind="ExternalInput").ap()
    kbias = nc.dram_tensor("kbias", [128, NKC], f32, kind="ExternalInput").ap()
    oT = nc.dram_tensor("oT", [E, S_q], f32, kind="ExternalOutput").ap()

    with tile.TileContext(nc) as tc, ExitStack() as ctx:
        cons = ctx.enter_context(tc.tile_pool(name="cons", bufs=1))
        wp = ctx.enter_context(tc.tile_pool(name="wp", bufs=1))
        acts = ctx.enter_context(tc.tile_pool(name="acts", bufs=1))
        pp = ctx.enter_context(tc.tile_pool(name="pp", bufs=12))
        ost = ctx.enter_context(tc.tile_pool(name="ost", bufs=4))
        nrm = ctx.enter_context(tc.tile_pool(name="nrm", bufs=1))

        # ---- constant/small loads ----
        bq2_t = cons.tile([128, NMC], f32, tag="bq2")
        bk2_t = cons.tile([128, NMC], f32, tag="bk2")
        kb_t = cons.tile([128, NKC], f32, tag="kb")
        ones2 = cons.tile([2, 64], bf16, tag="ones2")
        nc.sync.dma_start(bq2_t[:], bq2[:])
        nc.sync.dma_start(bk2_t[:], bk2[:])
        nc.sync.dma_start(kb_t[:], kbias[:])
        nc.vector.memset(ones2[:], 1.0)

        # ---- input loads in deadline order ----
        qkv = tc.tile_pool(name="qkv", bufs=1)
        inp = qkv.__enter__()
        wq_t = [wp.tile([128, HALF], bf16, tag=f"wq{e}", name=f"wq{e}") for e in range(NEC)]
        wk_t = [wp.tile([128, HALF], bf16, tag=f"wk{e}", name=f"wk{e}") for e in range(NEC)]
        wv_t = [wp.tile([128, HALF], bf16, tag=f"wv{e}", name=f"wv{e}") for e in range(NEC)]
        wo_t = [wp.tile([128, E], bf16, tag=f"wo{m}", name=f"wo{m}") for m in range(NMC)]
        kT_t = [inp.tile([128, S_pad], bf16, tag=f"kT{e}", name=f"kTt{e}") for e in range(NEC)]
        qTa_t = [inp.tile([128, QA], bf16, tag=f"qTa{e}", name=f"qTat{e}") for e in range(NEC)]
        qTb_t = [inp.tile([128, S_q - QA], bf16, tag=f"qTb{e}", name=f"qTbt{e}") for e in range(NEC)]
        vTa_t = [inp.tile([128, VA], bf16, tag=f"vTa{e}", name=f"vTat{e}") for e in range(NEC)]
        vTb_t = [inp.tile([128, S_pad - VA], bf16, tag=f"vTb{e}", name=f"vTbt{e}") for e in range(NEC)]
        for e in range(NEC):
            nc.sync.dma_start(wk_t[e][:], wk[128 * e:128 * (e + 1), :])
            nc.sync.dma_start(kT_t[e][:], kT[128 * e:128 * (e + 1), :])
        for e in range(NEC):
            nc.sync.dma_start(wq_t[e][:], wq[128 * e:128 * (e + 1), :])
            nc.sync.dma_start(qTa_t[e][:], qTa[128 * e:128 * (e + 1), :])
        for e in range(NEC):
            nc.sync.dma_start(wv_t[e][:], wv[128 * e:128 * (e + 1), :])
            nc.sync.dma_start(vTa_t[e][:], vTa[128 * e:128 * (e + 1), :])
        for e in range(NEC):
            nc.sync.dma_start(vTb_t[e][:], vTb[128 * e:128 * (e + 1), :])
        for e in range(NEC):
            nc.sync.dma_start(qTb_t[e][:], qTb[128 * e:128 * (e + 1), :])
        for m in range(NMC):
            nc.sync.dma_start(wo_t[m][:], wo[128 * m:128 * (m + 1), :])

        def qx(e, n0, nw):
            if n0 + nw <= QA:
                return qTa_t[e][:, n0:n0 + nw]
            return qTb_t[e][:, n0 - QA:n0 - QA + nw]

        def kx(e, n0, nw):
            return kT_t[e][:, n0:n0 + nw]

        def vx(e, j):  # key chunk j of vT
            if 128 * (j + 1) <= VA:
                return vTa_t[e][:, 128 * j:128 * (j + 1)]
            return vTb_t[e][:, 128 * j - VA:128 * (j + 1) - VA]

        kts = [acts.tile([128, S_pad], bf16, tag=f"kts{m}", name=f"kts{m}") for m in range(NMC)]
        qts = [acts.tile([128, S_q], bf16, tag=f"qts{m}", name=f"qts{m}") for m in range(NMC)]
        vhx = [acts.tile([128, 6, 128], bf16, tag=f"vhx{j}", name=f"vhx{j}") for j in range(NKC)]
        czT = [acts.tile([128, S_q], bf16, tag=f"czT{m}", name=f"czT{m}") for m in range(NMC)]

        def proj_mm(ps_, tagf, wt, xf, m, pair, e_lo, e_hi, pjs):
            if not pjs:
                pjs.extend(ps_.tile([128, 512], f32, tag=tagf(i), bufs=2,
                                    name=f"pj{m}_{pair[i][0]}_{id(wt)}")
                           for i in range(len(pair)))
            for e in range(e_lo, e_hi):
                for i, (n0, nw) in enumerate(pair):
                    nc.tensor.matmul(pjs[i][:, :nw],
                                     wt[e][:, 128 * m:128 * (m + 1)],
                                     xf(e, n0, nw),
                                     start=(e == 0), stop=(e == NEC - 1))

        def proj_evac(out, bias_t, m, pair, pjs):
            for i, (n0, nw) in enumerate(pair):
                nc.vector.tensor_scalar_add(out[m][:, n0:n0 + nw],
                                            pjs[i][:, :nw],
                                            bias_t[:, m:m + 1])

        def v_mm(ps_, tagf, chunks, e_lo, e_hi, pvs):
            if not pvs:
                pvs.extend(ps_.tile([128, 512], f32, tag=tagf(i), bufs=2,
                                    name=f"pv{chunks[i]}")
                           for i in range(len(chunks)))
            for e in range(e_lo, e_hi):
                for i, j in enumerate(chunks):
                    nc.tensor.matmul(pvs[i][:, 0:HALF], vx(e, j), wv_t[e][:],
                                     start=(e == 0), stop=(e == NEC - 1))

        def v_evac(chunks, pvs):
            for i, j in enumerate(chunks):
                nc.vector.memset(vhx[j][:, :, 64:128], 1.0)
                nc.vector.tensor_copy(
                    vhx[j][:, :, 0:64],
                    pvs[i][:, 0:HALF].rearrange("p (h d) -> p h d", h=6))

        # ---- prologue: K m0 n0-1 then Q m0 n0-1 (from the early tiles) ----
        kt2 = ntiles(S_pad)
        qt2 = ntiles(S_q)
        psp = tc.tile_pool(name="psp", bufs=1, space="PSUM")
        ps = psp.__enter__()
        pjs = []
        proj_mm(ps, lambda i: f"pj{i}", wk_t, kx, 0, kt2[0:2], 0, NEC, pjs)
        proj_evac(kts, bk2_t, 0, kt2[0:2], pjs)
        pjs = []
        proj_mm(ps, lambda i: f"pj{2 + i}", wq_t, qx, 0, qt2[0:2], 0, NEC, pjs)
        proj_evac(qts, bq2_t, 0, qt2[0:2], pjs)
        psp.__exit__(None, None, None)

        # ---- attention ----
        psa = tc.tile_pool(name="psa", bufs=1, space="PSUM")
        ps = psa.__enter__()

        deferred = []   # single FIFO: pops can never interleave an open
                        # accumulation because pushes only append
        pend_b = []     # normalization B-stages awaiting a 1-block lag
        hl_t = {}
        pushed_units = [0] * NQB

        def defer_proj(wt, xf, out, bias_t, m, pair, halves=True):
            pjs = []
            mid = NEC // 2 if halves else NEC

            def h1():
                proj_mm(ps, lambda i: "aux", wt, xf, m, pair, 0, mid, pjs)
                if not halves:
                    proj_evac(out, bias_t, m, pair, pjs)

            def h2():
                proj_mm(ps, lambda i: "aux", wt, xf, m, pair, mid, NEC, pjs)
                proj_evac(out, bias_t, m, pair, pjs)
            deferred.extend([h1, h2] if halves else [h1])

        def defer_v(chunks):
            pvs = []

            def h1():
                v_mm(ps, lambda i: "aux", chunks, 0, NEC // 2, pvs)

            def h2():
                v_mm(ps, lambda i: "aux", chunks, NEC // 2, NEC, pvs)
                v_evac(chunks, pvs)
            deferred.extend([h1, h2])

        # queue order == deadline order; see module docstring.
        for c0 in range(0, NKC, 2):
            defer_v(list(range(c0, min(c0 + 2, NKC))))
        if kt2[2:]:
            defer_proj(wk_t, kx, kts, bk2_t, 0, kt2[2:], halves=False)
        if qt2[2:4]:
            defer_proj(wq_t, qx, qts, bq2_t, 0, qt2[2:4])
        for m in range(1, NMC):
            defer_proj(wk_t, kx, kts, bk2_t, m, kt2[0:2])
            if kt2[2:]:
                defer_proj(wk_t, kx, kts, bk2_t, m, kt2[2:], halves=False)
            defer_proj(wq_t, qx, qts, bq2_t, m, qt2[0:2])
            if qt2[2:4]:
                defer_proj(wq_t, qx, qts, bq2_t, m, qt2[2:4])

        def make_unit_a(u, dn):
            def unit_a():
                # r = 1/den at partition 0; exact bf16 split r = hi + lo.
                rq = nrm.tile([1, QB], f32, tag="rq", bufs=4, name=f"rq{u}")
                nc.vector.reciprocal_approx_fast(rq[:], dn[:])
                hl = nrm.tile([1, QB], bf16, tag="hl", bufs=4, name=f"hl{u}")
                nc.vector.tensor_copy(hl[:], rq[:])
                lo = nrm.tile([1, QB], bf16, tag="lo", bufs=4, name=f"lo{u}")
                nc.vector.tensor_sub(lo[:], rq[:], hl[:])
                hl_t[u] = (hl, lo)
            return unit_a

        def make_unit_b(u, cs):
            qb, h = divmod(u, 6)
            m, half = divmod(h, 2)

            def unit_b():
                hl, lo = hl_t.pop(u)
                bcp = ps.tile([128, QB], f32, tag="aux", bufs=2,
                              name=f"bcp{u}")
                nc.tensor.matmul(bcp[0:64, :], ones2[0:1, :], hl[:],
                                 start=True, stop=False)
                nc.tensor.matmul(bcp[0:64, :], ones2[0:1, :], lo[:],
                                 start=False, stop=True)
                nc.vector.tensor_mul(
                    czT[m][64 * half:64 * (half + 1),
                           qb * QB:(qb + 1) * QB],
                    cs[0:64, :], bcp[0:64, :])
            return unit_b

        def make_oproj(qb):
            fns = []
            for e0 in range(0, NEC, 2):
                def fn(e0=e0, qb=qb):
                    ecs = (e0, e0 + 1)
                    pos = [ps.tile([128, QB], f32, tag="aux", bufs=2,
                                   name=f"po{qb}_{ec}") for ec in ecs]
                    for mm in range(NMC):
                        for j, ec in enumerate(ecs):
                            nc.tensor.matmul(
                                pos[j][:],
                                wo_t[mm][:, 128 * ec:128 * (ec + 1)],
                                czT[mm][:, qb * QB:(qb + 1) * QB],
                                start=(mm == 0), stop=(mm == NMC - 1))
                    for j, ec in enumerate(ecs):
                        ot = ost.tile([128, QB], f32, tag="ot",
                                      name=f"ot{qb}_{ec}")
                        nc.vector.tensor_copy(ot[:], pos[j][:])
                        nc.sync.dma_start(
                            oT[128 * ec:128 * (ec + 1),
                               qb * QB:(qb + 1) * QB], ot[:])
                fns.append(fn)
            return fns

        def evac(u, C):
            # context + denominator row out of PSUM; den row DMA'd to
            # partition 0 for the A-stage reciprocal.
            cs = nrm.tile([65, QB], f32, tag="cs", bufs=8, name=f"cs{u}")
            nc.vector.tensor_copy(cs[:], C[0:65, :])
            dn = nrm.tile([1, QB], f32, tag="dn", bufs=4, name=f"dn{u}")
            nc.sync.dma_start(dn[:], cs[64:65, :])
            deferred.append(make_unit_a(u, dn))
            pend_b.append((u, make_unit_b(u, cs)))

        def flush_b():
            # push last block's B-stages (their hi/lo is ready by now) and
            # any O-projection whose 6 units are all queued.
            while pend_b:
                u, fn = pend_b.pop(0)
                deferred.append(fn)
                qb = u // 6
                pushed_units[qb] += 1
                if pushed_units[qb] == 6:
                    deferred.extend(make_oproj(qb))

        pend_cs = []  # [(u, C), ...] awaiting evacuation
        nblk = 0
        for p in range(NMC):      # head pair: hA=2p (rows 0-63), hB=2p+1
            for qb in range(NQB):
                q0 = qb * QB
                hA, hB = 2 * p, 2 * p + 1
                CA = ps.tile([128, QB], f32, tag="CA", name=f"CA{qb}_{p}")
                CB = ps.tile([128, QB], f32, tag="CB", name=f"CB{qb}_{p}")
                flush_b()
                for pc in pend_cs:
                    evac(*pc)
                pend_cs = []

                def sc_pair(kc, St):
                    nc.tensor.matmul(
                        St[:, 0, :], kts[p][0:64, 128 * kc:128 * (kc + 1)],
                        qts[p][0:64, q0:q0 + QB],
                        start=True, stop=True, tile_position=(0, 0))
                    nc.tensor.matmul(
                        St[:, 1, :], kts[p][64:128, 128 * kc:128 * (kc + 1)],
                        qts[p][64:128, q0:q0 + QB],
                        start=True, stop=True, tile_position=(64, 0))

                S = ps.tile([128, 2, QB], f32, tag="S", bufs=2,
                            name=f"S{qb}_{p}_0")
                sc_pair(0, S)
                for kc in range(NKC):
                    # block 0 pops from kc0 and doubled: the V-projection
                    # fns must all EMIT before their first in-block readers
                    # (vhx[j] is read by the kc=j context matmul).
                    if (kc >= 1 or nblk == 0) and deferred:
                        deferred.pop(0)()
                        if ((nblk == 0 or (nblk >= 2 and len(deferred) > 14))
                                and deferred):
                            deferred.pop(0)()
                    S2 = None
                    if kc + 1 < NKC:
                        S2 = ps.tile([128, 2, QB], f32, tag="S", bufs=2,
                                     name=f"S{qb}_{p}_{kc + 1}")
                        sc_pair(kc + 1, S2)
                    P = pp.tile([128, 2, QB], bf16, tag="P",
                                name=f"P{qb}_{p}_{kc}")
                    nc.scalar.activation(P[:], S[:], FT.Exp,
                                         bias=kb_t[:, kc:kc + 1], scale=1.0)
                    nc.tensor.matmul(CA[:], vhx[kc][:, hA, :], P[:, 0, :],
                                     start=(kc == 0), stop=(kc == NKC - 1))
                    nc.tensor.matmul(CB[:], vhx[kc][:, hB, :], P[:, 1, :],
                                     start=(kc == 0), stop=(kc == NKC - 1))
                    S = S2
                pend_cs = [(qb * 6 + hA, CA), (qb * 6 + hB, CB)]
                nblk += 1

        # flush: evacuate last heads, push their B-stages, drain the queue
        flush_b()
        for pc in pend_cs:
            evac(*pc)
        flush_b()
        while deferred:
            deferred.pop(0)()
        psa.__exit__(None, None, None)
        qkv.__exit__(None, None, None)

    nc.compile()
    return nc


def _numpy_fallback(q, k, v, mask, Wq, bq, Wk, bk, Wv, bv, Wo, bo):
    B, Sq, _ = q.shape
    qh = (q @ Wq + bq).reshape(B, Sq, H, D).transpose(0, 2, 1, 3)
    kh = (k @ Wk + bk).reshape(B, -1, H, D).transpose(0, 2, 1, 3)
    vh = (v @ Wv + bv).reshape(B, -1, H, D).transpose(0, 2, 1, 3)
    s = np.einsum("bhqd,bhkd->bhqk", qh, kh) / np.sqrt(np.float32(D))
    s = s + np.where(mask == 0, np.float32(-1e9), np.float32(0))[:, None, None, :]
    s = s - s.max(-1, keepdims=True)
    w = np.exp(s)
    w = w / w.sum(-1, keepdims=True)
    ctx = np.einsum("bhqk,bhkd->bqhd", w, vh).reshape(B, Sq, E)
    return (ctx @ Wo + bo).astype(np.float32)


def kernel(q, k, v, mask, Wq, bq, Wk, bk, Wv, bv, Wo, bo):
    global _LAST
    q = np.asarray(q, np.float32)
    k = np.asarray(k, np.float32)
    v = np.asarray(v, np.float32)
    mask = np.asarray(mask)
    Wq = np.asarray(Wq, np.float32)
    bq = np.asarray(bq, np.float32)
    Wk = np.asarray(Wk, np.float32)
    bk = np.asarray(bk, np.float32)
    Wv = np.asarray(Wv, np.float32)
    bv = np.asarray(bv, np.float32)
    Wo = np.asarray(Wo, np.float32)
    bo = np.asarray(bo, np.float32)

    B, S_q, _ = q.shape
    idxs = [np.flatnonzero(mask[b]) for b in range(B)]
    ns = [len(ix) for ix in idxs]
    if (min(ns) == 0 or B * 2 != N_CORES or S_q % QB != 0
            or S_q < 2048 or max(ns) <= 512):
        return _numpy_fallback(q, k, v, mask, Wq, bq, Wk, bk, Wv, bv, Wo, bo)

    S_pad = ((max(ns) + 127) // 128) * 128
    NKC = S_pad // 128
    NMC = HALF // 128

    key = (S_q, S_pad)
    if key not in _CACHE:
        _CACHE[key] = _build(S_q, S_pad)
    nc = _CACHE[key]

    scale = np.float32(1.0 / np.sqrt(D))
    in_maps = []
    for c in range(N_CORES):
        b, j = divmod(c, 2)
        cols = slice(j * HALF, (j + 1) * HALF)
        kc_ = np.zeros((S_pad, E), np.float32)
        kc_[:ns[b]] = k[b][idxs[b]]
        vc_ = np.zeros((S_pad, E), np.float32)
        vc_[:ns[b]] = v[b][idxs[b]]
        kb_vec = np.zeros(S_pad, np.float32)
        kb_vec[ns[b]:] = -30000.0
        qT_ = np.ascontiguousarray(q[b].T).astype(bf16_np)
        vT_ = np.ascontiguousarray(vc_.T).astype(bf16_np)
        in_maps.append({
            "qTa": np.ascontiguousarray(qT_[:, 0:1024]),
            "qTb": np.ascontiguousarray(qT_[:, 1024:]),
            "kT": np.ascontiguousarray(kc_.T).astype(bf16_np),
            "vTa": np.ascontiguousarray(vT_[:, 0:512]),
            "vTb": np.ascontiguousarray(vT_[:, 512:]),
            "wq": (Wq[:, cols] * scale).astype(bf16_np),
            "wk": np.ascontiguousarray(Wk[:, cols]).astype(bf16_np),
            "wv": np.ascontiguousarray(Wv[:, cols]).astype(bf16_np),
            "wo": np.ascontiguousarray(Wo[cols, :]).astype(bf16_np),
            "bq2": np.ascontiguousarray((bq[cols] * scale).reshape(NMC, 128).T),
            "bk2": np.ascontiguousarray(bk[cols].reshape(NMC, 128).T),
            "kbias": np.ascontiguousarray(kb_vec.reshape(NKC, 128).T),
        })

    from concourse.bass_utils import run_bass_kernel_spmd
    res = run_bass_kernel_spmd(nc, in_maps, list(range(N_CORES)))
    _LAST = res

    bo_eff = bo + bv @ Wo
    out = np.empty((B, S_q, E), np.float32)
    for b in range(B):
        out[b] = (res.results[2 * b]["oT"] + res.results[2 * b + 1]["oT"]).T
        out[b] += bo_eff
    return out
